# revision 1
# baseline (speedup 1.0000x reference)
"""Trainium2 Bass kernel for nn_CustomCLIP (moe_routing).

Reference computation (B=16384, C=512, H=128, D=3, n_text=1380):
    h_d  = relu(x @ W1[d])                  for d in 0..2      [D,B,H]
    a_d  = relu(h_d @ W2[d])                                   [D,B,C]
    ad   = a[label[b], b, :]                (per-sample routing)
    f    = 0.2*ad + 0.8*x ; f /= ||f||
    t    = txt / ||txt||  (rows)
    out  = exp(ls) * f @ t.T                                   [B, n_text]

Strategy: batch data-parallel over 8 cores (2048 rows each). Host prep
(layout/scale only, no batch-scale compute): x is pre-transposed and
pre-scaled by 0.8 (W1 divided by 0.8 compensates; relu commutes with
positive scaling), W2 pre-scaled by 0.2, one-hot routing masks from the
labels, and text features row-normalized with exp(logit_scale) folded
in (O(NT*C), 0.004% of the FLOPs). The on-chip pipeline runs entirely
in "feature dim on partitions" orientation so no on-chip transposes are
needed anywhere:

    hT_d   = relu(W1'[d].T @ xT') * bcast(mask_d)      [128h, b]
    aT     = relu(sum_d W2'[d].T @ hmT_d)              [512c, b]
    fT     = aT + xT'                                  (= feats.T)
    nsq_b  = (sqsum fT slice).T @ ones -> [128b,·] per-partition norms
    logits = fT_slice.T @ ttn  scaled by 1/||f|| on the PSUM->SBUF copy

All matmuls use float32r (TF32-style full-rate fp32 mode); fp32r
operands are pre-rounded on the host (DMA inputs) or rounded on write
by the producing engine (declared fp32r out dtype). Mask broadcasts run
on GPSIMD (partition_broadcast); square pre-sums run on GPSIMD so the
DVE stays on the critical PSUM->SBUF copies. Per-chunk work is software
pipelined (A = adapter stages + norms, B = logits) with one-chunk DMA
lookahead.
"""

import numpy as np

import concourse.bass as bass
import concourse.bacc as bacc
import concourse.mybir as mybir
from concourse.tile import TileContext
from concourse.bass_utils import run_bass_kernel_spmd

F32 = mybir.dt.float32
F32R = mybir.dt.float32r
BF16 = mybir.dt.bfloat16
USE_BF16 = False
MM = BF16 if USE_BF16 else F32R
AF = mybir.ActivationFunctionType
ALU = mybir.AluOpType

B, C, H, D = 16384, 512, 128, 3
NT = (D + 1) * 345  # 1380
N_CORES = 8
BC = B // N_CORES   # 2048 rows per core
BF = 512            # batch free-dim chunk (one PSUM bank of fp32)
NCH = BC // BF      # 4 chunks per core
KC = C // 128       # 4 contraction chunks of 128
# text column ranges (<=512 per PSUM bank)
NRS = [(0, 512), (512, 1024), (1024, NT)]


def build_nc() -> bass.Bass:
    nc = bacc.Bacc(None, target_bir_lowering=False)

    # host sorts rows by domain: each core's shard spans <=2 domains, so
    # only that core's 2 experts (host-gathered weight pairs) are computed
    DK = 2
    xT = nc.dram_tensor("xt", [C, BC], MM, kind="ExternalInput")
    mk = nc.dram_tensor("mk", [1, DK, BC], F32, kind="ExternalInput")
    w1 = nc.dram_tensor("w1", [DK, C, H], MM, kind="ExternalInput")
    w2 = nc.dram_tensor("w2", [DK, H, C], MM, kind="ExternalInput")
    tt = nc.dram_tensor("tt", [C, NT], MM, kind="ExternalInput")
    out = nc.dram_tensor("out", [BC, NT], F32, kind="ExternalOutput")

    xT_r = xT[:].rearrange("(kc p) b -> p kc b", p=128)
    w1_r = w1[:].rearrange("d (kc p) h -> p (d kc) h", p=128)
    w2_r = w2[:].rearrange("d p c -> p d c")
    tt_r = tt[:].rearrange("(kc p) n -> p kc n", p=128)

    with TileContext(nc) as tc:
        with (
            nc.allow_low_precision(reason="fp32r (tf32) matmul operands"),
            tc.tile_pool(name="cpool", bufs=1) as cpool,
            tc.tile_pool(name="xpool", bufs=2) as xpool,
            tc.tile_pool(name="mpool", bufs=2) as mpool,
            tc.tile_pool(name="hpool", bufs=6) as hpool,
            tc.tile_pool(name="fpool", bufs=2) as fpool,
            tc.tile_pool(name="sqpool", bufs=2) as sqpool,
            tc.tile_pool(name="rnpool", bufs=4) as rnpool,
            tc.tile_pool(name="opool", bufs=3) as opool,
            tc.tile_pool(name="ps", bufs=3, space="PSUM") as ps,
            tc.tile_pool(name="psl", bufs=5, space="PSUM") as psl,
        ):
            # ---- constants; w1[d0] + chunk-0 x first so PE starts early ----
            w1_sb = cpool.tile([128, DK * KC, H], MM)
            nc.sync.dma_start(
                out=w1_sb[:, 0:KC, :], in_=w1_r[:, 0:KC, :]
            )

            def load_chunk(ch):
                b0 = ch * BF
                xt = xpool.tile([128, KC, BF], MM, tag="xt", name=f"xt_{ch}")
                for kc in range(KC):
                    nc.sync.dma_start(
                        out=xt[:, kc, :], in_=xT_r[:, kc, b0 : b0 + BF]
                    )
                mrow = mpool.tile([1, DK, BF], F32, tag="mrow", name=f"mrow_{ch}")
                nc.sync.dma_start(out=mrow, in_=mk[:, :, b0 : b0 + BF])
                return xt, mrow

            xt0, mrow0 = load_chunk(0)
            for d in range(1, DK):
                nc.sync.dma_start(
                    out=w1_sb[:, d * KC : (d + 1) * KC, :],
                    in_=w1_r[:, d * KC : (d + 1) * KC, :],
                )

            w2_sb = cpool.tile([128, DK, C], MM)
            nc.sync.dma_start(out=w2_sb, in_=w2_r)
            tt_sb = cpool.tile([128, KC, NT], MM)
            for kc in range(KC):
                nc.sync.dma_start(out=tt_sb[:, kc, :], in_=tt_r[:, kc, :])
            # memset cannot encode fp32r values; stage in f32 and cast-copy
            ones_st = cpool.tile([128, 2], F32)
            nc.vector.memset(ones_st, 1.0)
            ones2 = cpool.tile([128, 2], F32R)
            nc.vector.tensor_copy(ones2, ones_st)

            def part_a(ch, xt, mrow):
                """stages 1+2 and row norms for one batch chunk."""
                hms = []
                for d in range(DK):
                    mb = hpool.tile([128, BF], F32, tag="mb", name=f"mb_{ch}_{d}")
                    nc.gpsimd.partition_broadcast(mb, mrow[:, d, :])
                    h = ps.tile([128, BF], F32, tag="ps", name=f"h_{ch}_{d}")
                    for kc in range(KC):
                        nc.tensor.matmul(
                            h,
                            w1_sb[:, d * KC + kc, :],
                            xt[:, kc, :],
                            start=(kc == 0),
                            stop=(kc == KC - 1),
                        )
                    rh = hpool.tile([128, BF], F32, tag="rh", name=f"rh_{ch}_{d}")
                    nc.scalar.activation(out=rh, in_=h, func=AF.Relu)
                    hm = hpool.tile([128, BF], MM, tag="hm", name=f"hm_{ch}_{d}")
                    nc.vector.tensor_mul(hm, rh, mb)
                    hms.append(hm)

                f = fpool.tile([128, KC, BF], MM, tag="f", name=f"f_{ch}")
                for cc in range(KC):
                    a = ps.tile([128, BF], F32, tag="ps", name=f"a_{ch}_{cc}")
                    for d in range(DK):
                        nc.tensor.matmul(
                            a,
                            w2_sb[:, d, cc * 128 : (cc + 1) * 128],
                            hms[d],
                            start=(d == 0),
                            stop=(d == DK - 1),
                        )
                    ra = hpool.tile([128, BF], F32, tag="ra", name=f"ra_{ch}_{cc}")
                    nc.scalar.activation(out=ra, in_=a, func=AF.Relu)
                    nc.vector.tensor_add(f[:, cc, :], ra, xt[:, cc, :])

                # norms: square on ACT, pre-sum c-chunks on GPSIMD, then one
                # (K=128, N=2) fp32r matmul per 128-row subchunk into a
                # single [128, 8] PSUM bank; sqrt+recip once per chunk.
                sq = sqpool.tile([128, KC, BF], F32R, tag="sq", name=f"sq_{ch}")
                for cc in range(KC):
                    nc.scalar.activation(
                        out=sq[:, cc, :], in_=f[:, cc, :], func=AF.Square
                    )
                sqs = sqpool.tile([128, 2, BF], F32R, tag="sqs", name=f"sqs_{ch}")
                nc.gpsimd.tensor_add(sqs[:, 0, :], sq[:, 0, :], sq[:, 1, :])
                nc.gpsimd.tensor_add(sqs[:, 1, :], sq[:, 2, :], sq[:, 3, :])
                sqf = sqpool.tile([128, BF], F32R, tag="sqf", name=f"sqf_{ch}")
                nc.gpsimd.tensor_add(sqf, sqs[:, 0, :], sqs[:, 1, :])
                nsq = ps.tile([128, 4, 2], F32, tag="ps", name=f"nsq_{ch}")
                for b2 in range(4):
                    nc.tensor.matmul(
                        nsq[:, b2, :],
                        sqf[:, b2 * 128 : (b2 + 1) * 128],
                        ones2,
                        start=True,
                        stop=True,
                    )
                sn = rnpool.tile([128, 8], F32, tag="sn", name=f"sn_{ch}")
                nc.scalar.activation(out=sn, in_=nsq, func=AF.Sqrt)
                rn = rnpool.tile([128, 8], F32, tag="rn", name=f"rn_{ch}")
                nc.vector.reciprocal(out=rn, in_=sn)
                return f, rn

            def part_b(ch, f, rn):
                """logits matmuls + normalized copy + store for one chunk."""
                b0 = ch * BF
                for bs in range(4):
                    lps = []
                    for i, (n0, n1) in enumerate(NRS):
                        lp = psl.tile(
                            [128, 512], F32, tag="pl", name=f"lp_{ch}_{bs}_{i}"
                        )
                        lps.append(lp)
                    for kc in range(KC):
                        for i, (n0, n1) in enumerate(NRS):
                            nc.tensor.matmul(
                                lps[i][:, : n1 - n0],
                                f[:, kc, bs * 128 : (bs + 1) * 128],
                                tt_sb[:, kc, n0:n1],
                                start=(kc == 0),
                                stop=(kc == KC - 1),
                            )
                    ob = opool.tile([128, NT], F32, tag="ob", name=f"ob_{ch}_{bs}")
                    r0 = b0 + bs * 128
                    for i, (n0, n1) in enumerate(NRS):
                        nc.any.tensor_scalar_mul(
                            ob[:, n0:n1], lps[i][:, : n1 - n0],
                            rn[:, 2 * bs : 2 * bs + 1],
                        )
                        nc.sync.dma_start(
                            out=out[r0 : r0 + 128, n0:n1], in_=ob[:, n0:n1]
                        )

            # text features arrive host-normalized; chunk-0 logits can
            # stream right behind the text DMA. One-chunk lookahead on loads.
            f0, rn0 = part_a(0, xt0, mrow0)
            nxt = load_chunk(1)
            part_b(0, f0, rn0)
            for ch in range(1, NCH):
                xt, mrow = nxt
                f, rn = part_a(ch, xt, mrow)
                if ch + 1 < NCH:
                    nxt = load_chunk(ch + 1)
                part_b(ch, f, rn)

    nc.compile()
    return nc


_NC_CACHE: list = []


def _get_nc() -> bass.Bass:
    if not _NC_CACHE:
        _NC_CACHE.append(build_nc())
    return _NC_CACHE[0]


def _tf32_round(a: np.ndarray) -> np.ndarray:
    """Round fp32 to the fp32r/tf32 grid (10-bit mantissa, RNE)."""
    u = np.ascontiguousarray(a, dtype=np.float32).view(np.uint32)
    lsb = (u >> 13) & 1
    rounded = (u + 0x0FFF + lsb) & np.uint32(0xFFFFE000)
    return rounded.view(np.float32)


def make_in_maps(
    image_features: np.ndarray,
    domain_label: np.ndarray,
    W1: np.ndarray,
    W2: np.ndarray,
    text_features: np.ndarray,
    logit_scale: np.ndarray,
) -> list[dict[str, np.ndarray]]:
    x = np.asarray(image_features, dtype=np.float32)
    lab = np.asarray(domain_label).astype(np.int64)
    w1 = np.asarray(W1, dtype=np.float32)
    w2 = np.asarray(W2, dtype=np.float32)
    txt = np.asarray(text_features, dtype=np.float32)
    lsv = np.asarray(logit_scale, dtype=np.float32).reshape(1, 1)

    tn = txt / np.linalg.norm(txt, axis=1, keepdims=True)
    tn = tn * np.exp(lsv[0, 0])
    if USE_BF16:
        import ml_dtypes

        bf = ml_dtypes.bfloat16
        xT = np.ascontiguousarray((x * np.float32(0.8)).T).astype(bf)
        w1s = np.ascontiguousarray(w1 / np.float32(0.8)).astype(bf)
        w2s = np.ascontiguousarray(w2 * np.float32(0.2)).astype(bf)
        ttT = np.ascontiguousarray(tn.T).astype(bf)
    else:
        xT = _tf32_round((x * np.float32(0.8)).T)               # [C, B]
        w1s = _tf32_round(w1 / np.float32(0.8))
        w2s = _tf32_round(w2 * np.float32(0.2))
        ttT = _tf32_round(tn.T)                                 # [C, NT]
    # sort rows by domain so each core's contiguous shard spans <=2
    # domains; ship only that pair of experts to the core (same SPMD
    # program everywhere). xT columns follow the sort; output rows are
    # inverse-permuted on the host.
    perm = np.argsort(lab, kind="stable")
    lab_s = lab[perm]
    xT = np.ascontiguousarray(xT[:, perm])

    in_maps = []
    for c in range(N_CORES):
        sl = slice(c * BC, (c + 1) * BC)
        labc = lab_s[sl]
        doms = np.unique(labc)
        assert len(doms) <= 2, "core shard spans >2 domains"
        da = int(doms[0])
        db = int(doms[-1]) if len(doms) == 2 else da
        ma = (labc == da).astype(np.float32)
        mb = (labc == db).astype(np.float32) if db != da else np.zeros_like(ma)
        in_maps.append(
            {
                "xt": np.ascontiguousarray(xT[:, sl]),
                "mk": np.stack([ma, mb])[None],
                "w1": np.ascontiguousarray(w1s[[da, db]]),
                "w2": np.ascontiguousarray(w2s[[da, db]]),
                "tt": ttT,
            }
        )
    return in_maps, perm


def kernel(
    image_features: np.ndarray,
    domain_label: np.ndarray,
    W1: np.ndarray,
    W2: np.ndarray,
    text_features: np.ndarray,
    logit_scale: np.ndarray,
) -> np.ndarray:
    nc = _get_nc()
    in_maps, perm = make_in_maps(
        image_features, domain_label, W1, W2, text_features, logit_scale
    )
    res = run_bass_kernel_spmd(nc, in_maps, list(range(N_CORES)))
    out_sorted = np.concatenate([r["out"] for r in res.results], axis=0)
    out = np.empty_like(out_sorted)
    out[perm] = out_sorted
    return out

